# revision 1
# baseline (speedup 1.0000x reference)
"""Trainium2 Bass kernel for the KNet-style recurrent chain (batch=1).

Strategy (memory-bound problem, ~353MB of fp32 weights):
  - The small GRU chain + small FCs (~35MB) are REPLICATED on all 8 cores.
  - FC2 (the big Kalman-gain MLP: W2a [46080,1152], W2b [576,46080]) is
    tensor-parallel: each core gets 5760 rows of W2a and the matching 5760
    columns of W2b, computes a partial y [576]; the host sums the 8 partials
    and adds b2b (the "all-reduce" done on host).
  - Every matvec y = W @ x runs on the TensorEngine in WEIGHT-MOVING form:
        out[1, N] (+)= x_chunk[K, 1].T @ W.T_chunk[K, N]
    i.e. the tiny activation chunk is the stationary operand (fast fp32
    load) and the pre-transposed weights stream as the moving operand
    (~430ns per [128, 512] fp32 block, ~611 GB/s — above the per-core HBM
    rate).  Keeping weights stationary instead costs ~350ns per [128,128]
    tile (fp32 weight load), 3x too slow.
  - Matvec outputs live in free-layout [1, M] (one partition); elementwise
    GRU math happens there; PE transpose-mode matmuls ([1,128] -> [128,1],
    ~330ns) rebuild the partition-layout [128, ceil(d/128)] tiles consumed
    as the next layer's stationary chunks.
  - PSUM accumulation: start=True clears has_written for the WHOLE target
    bank, so it is set only on the first matmul into each bank; later
    first-writes to an element overwrite because has_written=0.
"""

import sys

sys.path.insert(0, "/opt/trn_rl_repo")

import numpy as np

NCORES = 8
H = 576                      # hidden size of all three GRUs
D2_HID, D2_IN, D2_OUT = 46080, 1152, 576
MSH = D2_HID // NCORES       # 5760 rows of W2a per core
NM2 = MSH // 128             # 45 output chunks per core
STRIPE = 512                 # FC2a output stripe width
W2B_GRP = 3                  # FC2b K-blocks per DMA

F32 = np.float32


def _ncols(d):
    return (d + 127) // 128


def _nsplits(m):
    """split free dim at 512 boundaries (= PSUM bank boundaries)."""
    return [(n0, min(512, m - n0)) for n0 in range(0, m, 512)]


_CACHE = {}


class _Vec:
    """An activation vector in SBUF P-layout [128, ncols]."""

    def __init__(self, tile, d):
        self.tile = tile
        self.d = d

    def chunks(self):
        for c in range(_ncols(self.d)):
            sz = min(128, self.d - c * 128)
            yield self.tile[0:sz, c : c + 1], sz


def _build_program(dbg=False):
    import concourse.bass as bass  # noqa: F401
    from concourse import bacc, mybir
    import concourse.tile as tile

    f32 = mybir.dt.float32
    f32r = mybir.dt.float32r
    AF = mybir.ActivationFunctionType

    nc = bacc.Bacc(
        "TRN2", target_bir_lowering=False, debug=False, num_devices=NCORES
    )

    def din(name, shape, dt=f32):
        return nc.dram_tensor(name, list(shape), dt, kind="ExternalInput")

    # --- dram inputs: activation vectors ---
    d_x5 = din("x5", (24, 1), f32r)
    d_x6 = din("x6", (24, 1), f32r)
    d_obs = din("obs", (48, 1), f32r)
    d_hq = din("h_q", (128, 5), f32r)      # P-layout (matvec operand)
    d_hsig = din("h_sig", (128, 5), f32r)
    d_hs = din("h_s", (128, 5), f32r)
    d_hq_f = din("h_q_f", (1, H))    # free-layout (elementwise operand)
    d_hsig_f = din("h_sig_f", (1, H))
    d_hs_f = din("h_s_f", (1, H))

    # --- dram inputs: weights, host-stored as W.T [K, M] row-major ---
    wshapes = {
        "w5": (24, 480), "w6": (24, 480), "w7": (48, 960), "w1": (576, 576),
        "wrz_q": (1056, 1152), "win_q": (480, 576), "whn_q": (576, 576),
        "wrz_sig": (1632, 1152), "win_sig": (1056, 576), "whn_sig": (576, 576),
        "wrz_s": (2112, 1152), "win_s": (1536, 576), "whn_s": (576, 576),
        "w2a": (D2_IN, MSH), "w2b": (MSH, D2_OUT),
    }
    dw = {k: din(k, v, f32r) for k, v in wshapes.items()}

    # --- dram inputs: biases in free-layout [1, M] ---
    bshapes = {
        "b5": 480, "b6": 480, "b7": 960, "b1": H,
        "brz_q": 1152, "bin_q": H, "bhn_q": H,
        "brz_sig": 1152, "bin_sig": H, "bhn_sig": H,
        "brz_s": 1152, "bin_s": H, "bhn_s": H,
    }
    db = {k: din(k, (1, v)) for k, v in bshapes.items()}

    d_b2a = din("b2a", (1, MSH))
    d_y = nc.dram_tensor("y", [1, D2_OUT], f32, kind="ExternalOutput")

    dbg_outs = {}

    def _dbg(name, tile_ap, shape):
        if not dbg:
            return
        dt = nc.dram_tensor(f"dbg_{name}", list(shape), f32,
                            kind="ExternalOutput")
        nc.sync.dma_start(out=dt[:], in_=tile_ap.bitcast(f32))
        dbg_outs[name] = dt

    with tile.TileContext(nc) as tc:
        with (
            tc.tile_pool(name="const", bufs=1) as constp,
            tc.tile_pool(name="vecs", bufs=1) as vecp,
            tc.tile_pool(name="smallw", bufs=3) as swp,
            tc.tile_pool(name="bigw", bufs=3) as bigp,
            tc.tile_pool(name="w2bp", bufs=2) as w2bp,
            tc.tile_pool(name="ps", bufs=1, space="PSUM") as psp,
        ):
            def load_const(dram, shape, name, dt=f32):
                t = constp.tile(list(shape), dt, name=name, tag=name)
                nc.sync.dma_start(out=t, in_=dram[:])
                return t

            x5 = _Vec(load_const(d_x5, (24, 1), "t_x5", f32r), 24)
            x6 = _Vec(load_const(d_x6, (24, 1), "t_x6", f32r), 24)
            obs = _Vec(load_const(d_obs, (48, 1), "t_obs", f32r), 48)
            h_q = _Vec(load_const(d_hq, (128, 5), "t_hq", f32r), H)
            h_sig = _Vec(load_const(d_hsig, (128, 5), "t_hsig", f32r), H)
            h_s = _Vec(load_const(d_hs, (128, 5), "t_hs", f32r), H)
            hf = {
                "q": load_const(d_hq_f, (1, H), "t_hq_f"),
                "sig": load_const(d_hsig_f, (1, H), "t_hsig_f"),
                "s": load_const(d_hs_f, (1, H), "t_hs_f"),
            }
            bt = {
                k: load_const(db[k], (1, v), "t_" + k)
                for k, v in bshapes.items()
            }
            ident = constp.tile([1, 1], f32, name="ident", tag="ident")
            nc.vector.memset(ident, 1.0)

            def load_w_chunks(wname, segs, m_out):
                """DMA pre-transposed weights; yield (wt_ap, rhs, ksz)."""
                w = dw[wname]
                chunks = []
                ro = 0
                # cap tile size at ~14KB/partition
                grp = max(1, 14336 // (m_out * 4))
                for v in segs:
                    nb, tail = v.d // 128, v.d % 128
                    rhs_cols = list(v.chunks())
                    for g0 in range(0, nb, grp):
                        gn = min(grp, nb - g0)
                        wt = swp.tile([128, gn, m_out], f32r, tag="sw",
                                      name=f"w_{wname}_{ro}f{g0}", bufs=3)
                        nc.sync.dma_start(
                            out=wt,
                            in_=w[ro + g0 * 128 : ro + (g0 + gn) * 128,
                                  :].rearrange("(b p) m -> p b m", p=128),
                        )
                        for b in range(gn):
                            chunks.append(
                                (wt[:, b, :], rhs_cols[g0 + b][0], 128)
                            )
                    if tail:
                        wtt = swp.tile([tail, m_out], f32r, tag="sw",
                                       name=f"w_{wname}_{ro}t", bufs=3)
                        nc.sync.dma_start(
                            out=wtt, in_=w[ro + nb * 128 : ro + v.d, :]
                        )
                        chunks.append((wtt, rhs_cols[nb][0], tail))
                    ro += v.d
                return chunks

            def matvec_f(wname, segs, m_out, bias_tile, act, out_name,
                         psum_tag, psum_bufs, out_tag=None, out_bufs=2):
                """free-layout matvec: returns sbuf AP [1, m_out] of
                act(W @ concat(segs) + b)."""
                psum = psp.tile([1, max(m_out, 1152)], f32,
                                name=f"ps_{out_name}", tag=psum_tag,
                                bufs=psum_bufs)
                chunks = load_w_chunks(wname, segs, m_out)
                nch = len(chunks)
                for ci, (wt_ap, rhs, ksz) in enumerate(chunks):
                    for n0, nsz in _nsplits(m_out):
                        nc.tensor.matmul(
                            psum[0:1, n0 : n0 + nsz],
                            rhs,
                            wt_ap[0:ksz, n0 : n0 + nsz],
                            start=(ci == 0),
                            stop=(ci == nch - 1),
                            skip_group_check=True,
                        )
                out = vecp.tile([1, m_out], f32, name=out_name,
                                tag=out_tag or out_name,
                                bufs=out_bufs if out_tag else 1)
                nc.vector.tensor_add(out, psum[0:1, 0:m_out], bias_tile)
                if act is not None:
                    nc.scalar.activation(out, out, act)
                return out

            def to_play(free_ap, d, name):
                """transpose free-layout [1, d] -> P-layout [128, ncols]."""
                n_m = _ncols(d)
                ps_t = psp.tile([128, NM2], f32, name=f"pst_{name}",
                                tag="tp", bufs=1)
                for c in range(n_m):
                    csz = min(128, d - c * 128)
                    nc.tensor.matmul(
                        ps_t[0:csz, c : c + 1],
                        free_ap[0:1, c * 128 : c * 128 + csz],
                        ident,
                        is_transpose=True,
                        start=(c == 0),
                        stop=(c == n_m - 1),
                        skip_group_check=True,
                    )
                pl = vecp.tile([128, n_m], f32r, name=name, tag=name)
                nc.vector.tensor_copy(pl, ps_t[:, 0:n_m])
                return _Vec(pl, d)

            def gru(g, x_segs, h, out_name):
                rz = matvec_f(f"wrz_{g}", x_segs + [h], 2 * H,
                              bt[f"brz_{g}"], AF.Sigmoid, f"rz_{g}",
                              "mv1", 1, out_tag="rz_sb")
                gin = matvec_f(f"win_{g}", x_segs, H, bt[f"bin_{g}"], None,
                               f"gin_{g}", "mv1", 1, out_tag="gin_sb")
                ghn = matvec_f(f"whn_{g}", [h], H, bt[f"bhn_{g}"], None,
                               f"ghn_{g}", "mv1", 1, out_tag="ghn_sb")
                # n = tanh(gin + r * ghn);  h' = n + z * (h - n)
                t3 = vecp.tile([1, H], f32, name=f"t3_{g}", tag="t3",
                                bufs=1)
                nc.vector.tensor_mul(t3, rz[0:1, 0:H], ghn)
                nc.vector.tensor_add(t3, gin, t3)
                n_t = vecp.tile([1, H], f32, name=f"n_{g}", tag="n_t",
                                bufs=1)
                nc.scalar.activation(n_t, t3, AF.Tanh)
                t5 = vecp.tile([1, H], f32, name=f"t5_{g}", tag="t5",
                                bufs=1)
                nc.vector.tensor_sub(t5, hf[g], n_t)
                nc.vector.tensor_mul(t5, rz[0:1, H : 2 * H], t5)
                hn = vecp.tile([1, H], f32, name=out_name, tag="hn",
                                bufs=1)
                nc.vector.tensor_add(hn, n_t, t5)
                return hn

            # ---- the chain ----
            out5_f = matvec_f("w5", [x5], 480, bt["b5"], AF.Relu,
                              "out5_f", "mv1", 1, out_tag="vf")
            out5 = to_play(out5_f, 480, "out5")
            _dbg("out5", out5.tile, (128, 4))
            hQ_f = gru("q", [out5], h_q, "hQ_f")
            hQ = to_play(hQ_f, H, "hQ")
            _dbg("hQ", hQ.tile, (128, 5))
            out6_f = matvec_f("w6", [x6], 480, bt["b6"], AF.Relu,
                              "out6_f", "mv1", 1, out_tag="vf")
            out6 = to_play(out6_f, 480, "out6")
            _dbg("out6", out6.tile, (128, 4))
            hSig_f = gru("sig", [hQ, out6], h_sig, "hSig_f")
            hSig = to_play(hSig_f, H, "hSig")
            _dbg("hSig", hSig.tile, (128, 5))
            out1_f = matvec_f("w1", [hSig], H, bt["b1"], AF.Relu,
                              "out1_f", "mv1", 1, out_tag="vf")
            out1 = to_play(out1_f, H, "out1")
            _dbg("out1", out1.tile, (128, 5))
            out7_f = matvec_f("w7", [obs], 960, bt["b7"], AF.Relu,
                              "out7_f", "mv1", 1, out_tag="vf")
            out7 = to_play(out7_f, 960, "out7")
            _dbg("out7", out7.tile, (128, 8))
            hS_f = gru("s", [out1, out7], h_s, "hS_f")
            if dbg:
                hS = to_play(hS_f, H, "hS")
                _dbg("hS", hS.tile, (128, 5))

            # ---- FC2a: h_fc = relu(W2a_shard @ [hSig, hS] + b2a_shard) ----
            # Build in2 = concat(hSig, hS) contiguously in free layout, then
            # transpose to a clean [128, 9] P-layout (1152 = 9*128 exactly).
            # Per output stripe of 512 the whole [1152, 512] weight block
            # arrives as ONE 2.36MB DMA.
            in2_f = vecp.tile([1, D2_IN], f32, name="in2_f", tag="in2_f")
            nc.vector.tensor_copy(in2_f[0:1, 0:H], hSig_f)
            nc.vector.tensor_copy(in2_f[0:1, H : 2 * H], hS_f)
            in2 = to_play(in2_f, D2_IN, "in2t")
            NK2 = D2_IN // 128  # 9
            ps_hfc = psp.tile([128, NM2], f32, name="ps_hfc", tag="tp",
                              bufs=1)
            n_tp = 0
            for m0, nsz in _nsplits(MSH):
                psf = psp.tile([1, STRIPE], f32, name=f"ps_f{m0}",
                               tag="fca", bufs=2)
                b2s = vecp.tile([1, STRIPE], f32, name=f"b2s_{m0}",
                                tag="b2as", bufs=2)
                nc.sync.dma_start(out=b2s[0:1, 0:nsz],
                                  in_=d_b2a[0:1, m0 : m0 + nsz])
                hstr = vecp.tile([1, STRIPE], f32, name=f"hstr_{m0}",
                                 tag="hstr", bufs=2)
                wt = bigp.tile([128, NK2, nsz], f32r, tag="w2a",
                               name=f"w2a_{m0}", bufs=3)
                nc.sync.dma_start(
                    out=wt,
                    in_=dw["w2a"][:, m0 : m0 + nsz].rearrange(
                        "(b p) m -> p b m", p=128
                    ),
                )
                rhs_cols = list(in2.chunks())
                for ci in range(NK2):
                    nc.tensor.matmul(
                        psf[0:1, 0:nsz],
                        rhs_cols[ci][0],
                        wt[:, ci, 0:nsz],
                        start=(ci == 0),
                        stop=(ci == NK2 - 1),
                        skip_group_check=True,
                    )
                # bias + relu into the free-layout accumulator
                nc.vector.tensor_add(
                    hstr[0:1, 0:nsz], psf[0:1, 0:nsz], b2s[0:1, 0:nsz]
                )
                nc.scalar.activation(
                    hstr[0:1, 0:nsz], hstr[0:1, 0:nsz], AF.Relu
                )
                # transpose this stripe into P-layout columns
                for c in range(nsz // 128):
                    col = m0 // 128 + c
                    nc.tensor.matmul(
                        ps_hfc[:, col : col + 1],
                        hstr[0:1, c * 128 : (c + 1) * 128],
                        ident,
                        is_transpose=True,
                        start=(n_tp == 0),
                        stop=(n_tp == NM2 - 1),
                        skip_group_check=True,
                    )
                    n_tp += 1
            h_fc = vecp.tile([128, NM2], f32r, name="h_fc", tag="h_fc")
            nc.vector.tensor_copy(h_fc, ps_hfc)
            _dbg("h_fc", h_fc, (128, NM2))

            # ---- FC2b: y_partial = W2b_shard @ h_fc  (out [1, 576]) ----
            ps512 = psp.tile([1, 512], f32, name="ps_y512", tag="y512",
                             bufs=1)
            ps64 = psp.tile([1, 64], f32, name="ps_y64", tag="y64", bufs=1)
            for g in range(NM2 // W2B_GRP):
                wt = w2bp.tile([128, W2B_GRP, D2_OUT], f32r, tag="w2b",
                               name=f"w2b_{g}", bufs=2)
                r0 = g * W2B_GRP * 128
                nc.sync.dma_start(
                    out=wt,
                    in_=dw["w2b"][r0 : r0 + W2B_GRP * 128, :].rearrange(
                        "(b p) m -> p b m", p=128
                    ),
                )
                for j in range(W2B_GRP):
                    kb = g * W2B_GRP + j
                    lhs = h_fc[:, kb : kb + 1]
                    nc.tensor.matmul(
                        ps512[0:1, :], lhs,
                        wt[:, j, 0:512],
                        start=(kb == 0), stop=(kb == NM2 - 1),
                        skip_group_check=True,
                    )
                    nc.tensor.matmul(
                        ps64[0:1, :], lhs,
                        wt[:, j, 512:576],
                        start=(kb == 0), stop=(kb == NM2 - 1),
                        skip_group_check=True,
                    )
            y_sb = constp.tile([1, D2_OUT], f32, name="y_sb", tag="y_sb")
            nc.vector.tensor_copy(y_sb[:, 0:512], ps512)
            nc.vector.tensor_copy(y_sb[:, 512:576], ps64)
            nc.sync.dma_start(out=d_y[:], in_=y_sb)

    nc.compile()
    return nc


def _get_program():
    if "nc" not in _CACHE:
        _CACHE["nc"] = _build_program()
    return _CACHE["nc"]


# ----------------------------------------------------------------------------
# host-side data prep
# ----------------------------------------------------------------------------


def _play(v, ncols):
    """length-d vector -> P-layout [128, ncols] (zero padded)."""
    v = np.asarray(v, F32).ravel()
    buf = np.zeros((ncols, 128), F32)
    buf.reshape(-1)[: v.size] = v
    return np.ascontiguousarray(buf.T)


def _prep_inputs(inputs):
    """Build the 8 per-core input maps from the full (unsharded) inputs."""
    g = {k: np.asarray(v, F32) for k, v in inputs.items()}

    common = {
        "x5": g["fw_evol_diff"].reshape(24, 1).copy(),
        "x6": g["fw_update_diff"].reshape(24, 1).copy(),
        "obs": np.concatenate(
            [g["obs_diff"], g["obs_innov_diff"]]
        ).reshape(48, 1).copy(),
        "h_q": _play(g["h_Q"], 5),
        "h_sig": _play(g["h_Sigma"], 5),
        "h_s": _play(g["h_S"], 5),
        "h_q_f": g["h_Q"].reshape(1, H).copy(),
        "h_sig_f": g["h_Sigma"].reshape(1, H).copy(),
        "h_s_f": g["h_S"].reshape(1, H).copy(),
        "w5": np.ascontiguousarray(g["W5"].T),
        "w6": np.ascontiguousarray(g["W6"].T),
        "w7": np.ascontiguousarray(g["W7"].T),
        "w1": np.ascontiguousarray(g["W1"].T),
        "b5": g["b5"].reshape(1, -1).copy(),
        "b6": g["b6"].reshape(1, -1).copy(),
        "b7": g["b7"].reshape(1, -1).copy(),
        "b1": g["b1"].reshape(1, -1).copy(),
    }
    for tag, suf in (("q", "Q"), ("sig", "Sig"), ("s", "S")):
        Wih, Whh = g[f"Wih_{suf}"], g[f"Whh_{suf}"]
        bih, bhh = g[f"bih_{suf}"], g[f"bhh_{suf}"]
        common[f"wrz_{tag}"] = np.ascontiguousarray(
            np.concatenate([Wih[0 : 2 * H], Whh[0 : 2 * H]], axis=1).T
        )
        common[f"win_{tag}"] = np.ascontiguousarray(Wih[2 * H :].T)
        common[f"whn_{tag}"] = np.ascontiguousarray(Whh[2 * H :].T)
        common[f"brz_{tag}"] = (bih[0 : 2 * H] + bhh[0 : 2 * H]).reshape(1, -1)
        common[f"bin_{tag}"] = bih[2 * H :].reshape(1, -1).copy()
        common[f"bhn_{tag}"] = bhh[2 * H :].reshape(1, -1).copy()

    in_maps = []
    for k in range(NCORES):
        m = dict(common)
        sl = slice(k * MSH, (k + 1) * MSH)
        m["w2a"] = np.ascontiguousarray(g["W2a"][sl, :].T)
        m["w2b"] = np.ascontiguousarray(g["W2b"][:, sl].T)
        m["b2a"] = g["b2a"][sl].reshape(1, -1).copy()
        in_maps.append(m)
    return in_maps


def run(trace=False, **inputs):
    from concourse.bass_utils import run_bass_kernel_spmd

    nc = _get_program()
    in_maps = _prep_inputs(inputs)
    res = run_bass_kernel_spmd(nc, in_maps, list(range(NCORES)), trace=trace)
    y = np.zeros(D2_OUT, np.float64)
    for r in res.results:
        y += r["y"].reshape(-1).astype(np.float64)
    out = (y.astype(F32) + np.asarray(inputs["b2b"], F32)).reshape(24, 24)
    return out, res


def kernel(**inputs):
    out, _ = run(trace=False, **inputs)
    return out



# revision 9
# speedup vs baseline: 2.1270x; 2.1270x over previous
"""Trainium2 Bass kernel for the KNet-style recurrent chain (batch=1), v2.

Strategy (memory-bound, ~353MB fp32 weights on host):
  - All weights cast to bf16 on host and PRE-SWIZZLED into the exact SBUF
    tile layout [128, nk, nsz] so every weight DMA is one fully
    contiguous flat copy.
  - FC2 tensor-parallel across 8 cores (5760 rows of W2a + matching
    columns of W2b per core); host sums the 8 partial y's + b2b.
  - GRU chain + small FCs replicated on all cores.
  - Matvecs run weight-STREAMING on the PE: out[1,N] += x[128,1].T @
    W[128,N<=512]; x-chunks are bf16 stationary columns.
  - Every activation vector lives in P-layout [128, ceil(d/128)] with
    128-padded segments; all biases are FOLDED INTO THE WEIGHTS via
    designated always-1.0 pad slots (slot = element M of each padded
    vector).  Input vectors get the 1.0 from the host; FC outputs get it
    via relu(1*1); GRU outputs h' get it via a sigmoid(30)=1 entry in
    the z-gate pad column (h'[M] = z[M]*h[M] = 1*1).
  - Matvec psum [1,<=512] stripes drain on ACT (fused sigmoid/relu) or
    DVE (copy) into bf16 free-layout rows; PE transposes rebuild
    P-layout; GRU pointwise runs on [128,5] P-layout tiles (fast DVE).
  - FC2a weight stripes are fully SBUF-RESIDENT: their 12 DMAs are woven
    between the GRU stages so HBM streams continuously from t=0;
    FC2b streams through a 3-buffer ring on the scalar DMA queue,
    interleaved with FC2a compute.
"""

import sys

sys.path.insert(0, "/opt/trn_rl_repo")

import numpy as np
import ml_dtypes

BF16 = ml_dtypes.bfloat16
F32 = np.float32

NCORES = 8
H = 576
D2_HID, D2_IN, D2_OUT = 46080, 1152, 576
MSH = D2_HID // NCORES       # 5760 FC2-hidden rows per core
NM2 = MSH // 128             # 45 h_fc columns per core
FCB_GRP = 3                  # FC2b k-blocks per DMA group

# matvec specs: name -> (seg names, Kp, Mp, act)
#   segs are padded to whole 128-cols; Kp = sum of padded seg lengths.
MV = {
    "fc5":  (["x5"], 128, 512, "relu"),
    "q_rz": (["out5", "h_q"], 1152, 1280, "sigmoid"),
    "q_in": (["out5"], 512, 640, None),
    "q_hn": (["h_q"], 640, 640, None),
    "fc6":  (["x6"], 128, 512, "relu"),
    "sig_rz": (["hQ", "out6", "h_sig"], 1792, 1280, "sigmoid"),
    "sig_in": (["hQ", "out6"], 1152, 640, None),
    "sig_hn": (["h_sig"], 640, 640, None),
    "fc1":  (["hSig"], 640, 640, "relu"),
    "fc7":  (["obs"], 128, 1024, "relu"),
    "s_rz": (["out1", "out7", "h_s"], 2304, 1280, "sigmoid"),
    "s_in": (["out1", "out7"], 1664, 640, None),
    "s_hn": (["h_s"], 640, 640, None),
}
SEG_COLS = {
    "x5": 1, "x6": 1, "obs": 1,
    "out5": 4, "out6": 4, "out7": 8, "out1": 5,
    "h_q": 5, "h_sig": 5, "h_s": 5,
    "hQ": 5, "hSig": 5, "hS": 5,
}

_CACHE = {}


def _stripes(mp):
    return [(n0, min(512, mp - n0)) for n0 in range(0, mp, 512)]


def _build_program():
    import concourse.bass as bass  # noqa: F401
    from concourse import bacc, mybir
    import concourse.tile as tile

    f32 = mybir.dt.float32
    bf16 = mybir.dt.bfloat16
    AF = mybir.ActivationFunctionType

    nc = bacc.Bacc(
        "TRN2", target_bir_lowering=False, debug=False, num_devices=NCORES
    )

    def din(name, shape, dt=bf16):
        return nc.dram_tensor(name, list(shape), dt, kind="ExternalInput")

    # dram inputs
    d_const = {
        "x5": din("x5", (128, 1)), "x6": din("x6", (128, 1)),
        "obs": din("obs", (128, 1)),
        "h_q": din("h_q", (128, 5)), "h_sig": din("h_sig", (128, 5)),
        "h_s": din("h_s", (128, 5)),
    }
    d_w = {}
    for name, (segs, kp, mp, act) in MV.items():
        nk = kp // 128
        for si, (n0, nsz) in enumerate(_stripes(mp)):
            d_w[f"{name}_s{si}"] = din(f"{name}_s{si}", (128, nk, nsz))
    for si, (n0, nsz) in enumerate(_stripes(MSH)):
        d_w[f"fc2a_s{si}"] = din(f"fc2a_s{si}", (128, 10, nsz))
    for g in range(NM2 // FCB_GRP):
        d_w[f"fc2b_g{g}"] = din(f"fc2b_g{g}", (128, FCB_GRP, D2_OUT))
    d_y = nc.dram_tensor("y", [1, D2_OUT], f32, kind="ExternalOutput")

    with tile.TileContext(nc) as tc:
        with (
            tc.tile_pool(name="const", bufs=1) as constp,
            tc.tile_pool(name="vecs", bufs=1) as vecp,
            tc.tile_pool(name="rows", bufs=1) as rowp,
            tc.tile_pool(name="gw", bufs=3) as gwp,
            tc.tile_pool(name="fc2a", bufs=1) as fc2ap,
            tc.tile_pool(name="w2bp", bufs=3) as w2bp,
            tc.tile_pool(name="ps", bufs=1, space="PSUM") as psp,
        ):
            segs = {}

            def load_const(key, ncols):
                t = constp.tile([128, ncols], bf16, name=f"t_{key}",
                                tag=f"t_{key}")
                nc.sync.dma_start(out=t, in_=d_const[key][:])
                segs[key] = t

            for k in ("x5", "x6", "obs"):
                load_const(k, 1)
            for k in ("h_q", "h_sig", "h_s"):
                load_const(k, 5)
            ident = constp.tile([1, 1], bf16, name="ident", tag="ident")
            nc.vector.memset(ident, 1.0)

            # FC2a resident stripe tiles; DMAs woven between GRU stages.
            fc2a_tiles = []
            _fc2a_pending = list(enumerate(_stripes(MSH)))

            def drop_fc2a(n):
                for _ in range(n):
                    if not _fc2a_pending:
                        return
                    si, (n0, nsz) = _fc2a_pending.pop(0)
                    t = fc2ap.tile([128, 10, nsz], bf16,
                                   name=f"w_fc2a_{si}", tag=f"fc2a_{si}")
                    nc.sync.dma_start(out=t, in_=d_w[f"fc2a_s{si}"][:])
                    fc2a_tiles.append(t)

            def seg_cols(names):
                cols = []
                for s in names:
                    for j in range(SEG_COLS[s]):
                        cols.append((segs[s], j))
                return cols

            def emit_matvec(name):
                """MMs + drains for one matvec; returns bf16 row [1, Mp]."""
                seg_names, kp, mp, act = MV[name]
                nk = kp // 128
                cols = seg_cols(seg_names)
                assert len(cols) == nk, (name, len(cols), nk)
                row = rowp.tile([1, mp], bf16, name=f"row_{name}",
                                tag=f"row_{name}")
                for si, (n0, nsz) in enumerate(_stripes(mp)):
                    wt = gwp.tile([128, nk, nsz], bf16, tag="gw",
                                  name=f"w_{name}_{si}", bufs=3)
                    nc.sync.dma_start(out=wt, in_=d_w[f"{name}_s{si}"][:])
                    ps = psp.tile([1, 512], f32, tag="mv", bufs=3,
                                  name=f"ps_{name}_{si}")
                    for c, (st, j) in enumerate(cols):
                        nc.tensor.matmul(
                            ps[0:1, 0:nsz], st[:, j : j + 1],
                            wt[:, c, 0:nsz],
                            start=(c == 0), stop=(c == nk - 1),
                            skip_group_check=True,
                        )
                    dst = row[0:1, n0 : n0 + nsz]
                    if act == "sigmoid":
                        nc.scalar.activation(dst, ps[0:1, 0:nsz], AF.Sigmoid)
                    elif act == "relu":
                        nc.scalar.activation(dst, ps[0:1, 0:nsz], AF.Relu)
                    else:
                        nc.vector.tensor_copy(dst, ps[0:1, 0:nsz])
                return row

            def emit_transposes(tp, row, ncols, col0, n_done, n_total):
                """row[1, ncols*128] -> tp[:, col0:col0+ncols, 0].

                tp is [128, n, 2] bf16 so every written column starts on a
                4-byte PSUM boundary (odd bf16 columns are illegal).
                """
                for c in range(ncols):
                    nc.tensor.matmul(
                        tp[:, col0 + c, 0:1],
                        row[0:1, c * 128 : (c + 1) * 128], ident,
                        is_transpose=True,
                        start=(n_done + c == 0),
                        stop=(n_done + c == n_total - 1),
                        skip_group_check=True,
                    )
                return n_done + ncols

            def do_fc(name, out_name):
                row = emit_matvec(name)
                mp = MV[name][2]
                nc_ = mp // 128
                tp = psp.tile([128, 20, 2], bf16, tag="tp", bufs=2,
                              name=f"tp_{name}")
                emit_transposes(tp, row, nc_, 0, 0, nc_)
                out = vecp.tile([128, nc_], bf16, name=out_name,
                                tag=out_name)
                nc.vector.tensor_copy(out, tp[:, 0:nc_, 0])
                segs[out_name] = out

            def do_gru(g, h_name, out_name):
                row_rz = emit_matvec(f"{g}_rz")
                row_in = emit_matvec(f"{g}_in")
                row_hn = emit_matvec(f"{g}_hn")
                tp = psp.tile([128, 20, 2], bf16, tag="tp", bufs=2,
                              name=f"tp_{g}")
                n = emit_transposes(tp, row_rz, 10, 0, 0, 20)
                n = emit_transposes(tp, row_in, 5, 10, n, 20)
                emit_transposes(tp, row_hn, 5, 15, n, 20)
                rzc = vecp.tile([128, 10], bf16, name=f"rzc_{g}", tag="rzc",
                                bufs=2)
                gh = vecp.tile([128, 10], bf16, name=f"gh_{g}", tag="gh",
                               bufs=2)
                nc.vector.tensor_copy(rzc, tp[:, 0:10, 0])
                nc.vector.tensor_copy(gh, tp[:, 10:20, 0])
                t1 = vecp.tile([128, 5], f32, name=f"t1_{g}", tag="t1",
                               bufs=2)
                nt = vecp.tile([128, 5], f32, name=f"nt_{g}", tag="nt",
                               bufs=2)
                t3 = vecp.tile([128, 5], f32, name=f"t3_{g}", tag="t3",
                               bufs=2)
                h_tile = segs[h_name]
                nc.vector.tensor_mul(t1, rzc[:, 0:5], gh[:, 5:10])
                nc.vector.tensor_add(t1, gh[:, 0:5], t1)
                nc.scalar.activation(nt, t1, AF.Tanh)
                nc.vector.tensor_sub(t3, h_tile, nt)
                nc.vector.tensor_mul(t3, rzc[:, 5:10], t3)
                out = vecp.tile([128, 5], bf16, name=out_name, tag=out_name)
                nc.vector.tensor_add(out, nt, t3)
                segs[out_name] = out

            # ---- the chain ----
            do_fc("fc5", "out5")
            drop_fc2a(2)
            do_gru("q", "h_q", "hQ")
            drop_fc2a(3)
            do_fc("fc6", "out6")
            drop_fc2a(1)
            do_gru("sig", "h_sig", "hSig")
            drop_fc2a(2)
            do_fc("fc1", "out1")
            do_fc("fc7", "out7")
            drop_fc2a(2)
            do_gru("s", "h_s", "hS")
            drop_fc2a(2)
            assert not _fc2a_pending

            # ---- FC2a + woven FC2b ----
            in2_cols = seg_cols(["hSig", "hS"])
            ps_hfc = psp.tile([128, NM2, 2], bf16, tag="hfc", bufs=1,
                              name="ps_hfc")
            h_fc = vecp.tile([128, NM2], bf16, name="h_fc", tag="h_fc")
            ps_y = psp.tile([1, D2_OUT], f32, tag="y", bufs=1, name="ps_y")
            ntp = 0
            fcb_done = 0
            for si, (n0, nsz) in enumerate(_stripes(MSH)):
                wt = fc2a_tiles[si]
                ps = psp.tile([1, 512], f32, tag="mv", bufs=3,
                              name=f"ps_fc2a_{si}")
                for c, (st, j) in enumerate(in2_cols):
                    nc.tensor.matmul(
                        ps[0:1, 0:nsz], st[:, j : j + 1], wt[:, c, 0:nsz],
                        start=(c == 0), stop=(c == 9),
                        skip_group_check=True,
                    )
                hstr = rowp.tile([1, 512], bf16, name=f"hstr_{si}",
                                 tag="hstr", bufs=2)
                nc.scalar.activation(hstr[0:1, 0:nsz], ps[0:1, 0:nsz],
                                     AF.Relu)
                nb = nsz // 128
                for c in range(nb):
                    nc.tensor.matmul(
                        ps_hfc[:, 4 * si + c, 0:1],
                        hstr[0:1, c * 128 : (c + 1) * 128], ident,
                        is_transpose=True,
                        start=(ntp == 0), stop=(ntp == NM2 - 1),
                        skip_group_check=True,
                    )
                    ntp += 1
                nc.vector.tensor_copy(
                    h_fc[:, 4 * si : 4 * si + nb],
                    ps_hfc[:, 4 * si : 4 * si + nb, 0],
                )
                navail = 4 * si + nb
                while (fcb_done + 1) * FCB_GRP <= navail:
                    gidx = fcb_done
                    wtb = w2bp.tile([128, FCB_GRP, D2_OUT], bf16, tag="w2b",
                                    name=f"w2b_{gidx}", bufs=3)
                    nc.scalar.dma_start(out=wtb, in_=d_w[f"fc2b_g{gidx}"][:])
                    for j in range(FCB_GRP):
                        kb = gidx * FCB_GRP + j
                        lhs = h_fc[:, kb : kb + 1]
                        nc.tensor.matmul(
                            ps_y[0:1, 0:512], lhs, wtb[:, j, 0:512],
                            start=(kb == 0), stop=(kb == NM2 - 1),
                            skip_group_check=True,
                        )
                        nc.tensor.matmul(
                            ps_y[0:1, 512:576], lhs, wtb[:, j, 512:576],
                            start=(kb == 0), stop=(kb == NM2 - 1),
                            skip_group_check=True,
                        )
                    fcb_done += 1
            assert fcb_done == NM2 // FCB_GRP

            y_sb = constp.tile([1, D2_OUT], f32, name="y_sb", tag="y_sb")
            nc.vector.tensor_copy(y_sb, ps_y[0:1, 0:D2_OUT])
            nc.sync.dma_start(out=d_y[:], in_=y_sb)

    nc.compile()
    return nc


def _get_program():
    if "nc" not in _CACHE:
        _CACHE["nc"] = _build_program()
    return _CACHE["nc"]


# ----------------------------------------------------------------------------
# host-side data prep
# ----------------------------------------------------------------------------


def _play(v, ncols, one_slot=None):
    """vector -> P-layout [128, ncols] bf16, zero padded, optional 1.0 slot."""
    v = np.asarray(v, F32).ravel()
    buf = np.zeros((ncols, 128), F32)
    buf.reshape(-1)[: v.size] = v
    if one_slot is not None:
        buf.reshape(-1)[one_slot] = 1.0
    return np.ascontiguousarray(buf.T).astype(BF16)


def _rz_cols(w):
    """[k, 1152] (r|z) -> [k, 1280] with 640-aligned r and z sections."""
    out = np.zeros((w.shape[0], 1280), F32)
    out[:, 0:576] = w[:, 0:576]
    out[:, 640:1216] = w[:, 576:1152]
    return out


def _swizzle(m, wp, kp, mp):
    """padded fp32 [Kp, Mp] -> per-stripe contiguous bf16 [128, nk, nsz]."""
    nk = kp // 128
    w3 = wp.reshape(nk, 128, mp).transpose(1, 0, 2)
    out = {}
    for si, (n0, nsz) in enumerate(_stripes(mp)):
        out[f"{m}_s{si}"] = np.ascontiguousarray(
            w3[:, :, n0 : n0 + nsz].astype(BF16)
        )
    return out


def _gru_host(tag, g, Wih, Whh, bih, bhh, xsegs):
    """Build the padded rz/in/hn weight matrices for one GRU.

    xsegs: list of (rows_in_Wih, padded_len) for the x-part segments.
    """
    out = {}
    kp_rz = sum(p for _, p in xsegs) + 640
    wp = np.zeros((kp_rz, 1280), F32)
    r0 = 0   # row offset into Wih's input dim
    p0 = 0   # row offset into the padded layout
    for rows, pad in xsegs:
        wp[p0 : p0 + rows] = _rz_cols(Wih[0:1152, r0 : r0 + rows].T)
        r0 += rows
        p0 += pad
    wp[p0 : p0 + 576] = _rz_cols(Whh[0:1152].T)
    brow = _rz_cols((bih[0:1152] + bhh[0:1152])[None, :])
    brow[0, 1216] = 30.0     # sigmoid(30)=1 -> h'[576]=1 (bias slot)
    wp[p0 + 576] = brow
    out.update(_swizzle(f"{tag}_rz", wp, kp_rz, 1280))

    kp_in = sum(p for _, p in xsegs)
    wp = np.zeros((kp_in, 640), F32)
    r0 = p0 = 0
    for rows, pad in xsegs:
        wp[p0 : p0 + rows, 0:576] = Wih[1152:1728, r0 : r0 + rows].T
        r0 += rows
        p0 += pad
    # bias at the first x-segment's 1-slot (= row xsegs[0][0])
    wp[xsegs[0][0], 0:576] = bih[1152:1728]
    out.update(_swizzle(f"{tag}_in", wp, kp_in, 640))

    wp = np.zeros((640, 640), F32)
    wp[0:576, 0:576] = Whh[1152:1728].T
    wp[576, 0:576] = bhh[1152:1728]
    out.update(_swizzle(f"{tag}_hn", wp, 640, 640))
    return out


def _fc_host(tag, W, b, kreal, kp, mp, one_col=None):
    """FC weight: wp[0:kreal] = W.T, bias row at kreal, optional 1.0."""
    wp = np.zeros((kp, mp), F32)
    m = W.shape[0]
    wp[0:kreal, 0:m] = W.T
    wp[kreal, 0:m] = b
    if one_col is not None:
        wp[kreal, one_col] = 1.0
    return _swizzle(tag, wp, kp, mp)


def _prep_inputs(inputs):
    g = {k: np.asarray(v, F32) for k, v in inputs.items()}

    common = {
        "x5": _play(g["fw_evol_diff"], 1, one_slot=24),
        "x6": _play(g["fw_update_diff"], 1, one_slot=24),
        "obs": _play(
            np.concatenate([g["obs_diff"], g["obs_innov_diff"]]), 1,
            one_slot=48,
        ),
        "h_q": _play(g["h_Q"], 5, one_slot=576),
        "h_sig": _play(g["h_Sigma"], 5, one_slot=576),
        "h_s": _play(g["h_S"], 5, one_slot=576),
    }
    common.update(_fc_host("fc5", g["W5"], g["b5"], 24, 128, 512,
                           one_col=480))
    common.update(_fc_host("fc6", g["W6"], g["b6"], 24, 128, 512,
                           one_col=480))
    common.update(_fc_host("fc7", g["W7"], g["b7"], 48, 128, 1024,
                           one_col=960))
    common.update(_fc_host("fc1", g["W1"], g["b1"], 576, 640, 640,
                           one_col=576))
    common.update(_gru_host("q", "q", g["Wih_Q"], g["Whh_Q"], g["bih_Q"],
                            g["bhh_Q"], [(480, 512)]))
    common.update(_gru_host("sig", "sig", g["Wih_Sig"], g["Whh_Sig"],
                            g["bih_Sig"], g["bhh_Sig"],
                            [(576, 640), (480, 512)]))
    common.update(_gru_host("s", "s", g["Wih_S"], g["Whh_S"], g["bih_S"],
                            g["bhh_S"], [(576, 640), (960, 1024)]))

    w2aT = g["W2a"].T    # [1152, 46080]
    w2bT = g["W2b"].T    # [46080, 576]
    in_maps = []
    for k in range(NCORES):
        m = dict(common)
        sl = slice(k * MSH, (k + 1) * MSH)
        wp = np.zeros((1280, MSH), F32)
        wp[0:576] = w2aT[0:576, sl]
        wp[576] = g["b2a"][sl]
        wp[640:1216] = w2aT[576:1152, sl]
        m.update(_swizzle("fc2a", wp, 1280, MSH))
        wb3 = w2bT[sl].reshape(NM2, 128, D2_OUT).transpose(1, 0, 2)
        for gi in range(NM2 // FCB_GRP):
            m[f"fc2b_g{gi}"] = np.ascontiguousarray(
                wb3[:, gi * FCB_GRP : (gi + 1) * FCB_GRP, :].astype(BF16)
            )
        in_maps.append(m)
    return in_maps


def run(trace=False, **inputs):
    from concourse.bass_utils import run_bass_kernel_spmd

    nc = _get_program()
    in_maps = _prep_inputs(inputs)
    res = run_bass_kernel_spmd(nc, in_maps, list(range(NCORES)), trace=trace)
    y = np.zeros(D2_OUT, np.float64)
    for r in res.results:
        y += r["y"].reshape(-1).astype(np.float64)
    out = (y.astype(F32) + np.asarray(inputs["b2b"], F32)).reshape(24, 24)
    return out, res


def kernel(**inputs):
    out, _ = run(trace=False, **inputs)
    return out


# revision 11
# speedup vs baseline: 2.1563x; 1.0138x over previous
"""Trainium2 Bass kernel for the KNet-style recurrent chain (batch=1), v3.

Distribution: FC2 tensor-parallel across 8 cores (5760 W2a rows + the
matching W2b columns per core; host sums the 8 partial y's + b2b); the
small GRU/FC chain is replicated on every core.

Memory strategy (the kernel is DMA-bound):
  - GRU/small-FC weights in fp8 e4m3, scaled x32 on host (their ~0.02
    magnitudes would land in e4m3's subnormal range unscaled); the 1/32
    descale is folded into each drain (ACT scale= / DVE scalar-mul).
    Activations stay bf16 (mixed bf16xfp8 matmul).  FC2 stays bf16
    (fp8 FC2 fails the 2e-2 accuracy gate).
  - All weights PRE-SWIZZLED on host into the exact SBUF tile layout
    [128, nk, nsz] so every weight DMA is one contiguous flat copy.
  - All biases are FOLDED INTO THE WEIGHTS via designated always-1.0
    pad slots (slot = element M of each 128-padded vector).  Inputs get
    the 1.0 from the host; FC outputs via relu(1*1); GRU outputs via a
    sigmoid(14)=1 entry in the z-gate pad column (h'[576] = z*h = 1).
  - Matvec psums [1,<=512] drain on ACT (fused sigmoid/relu + descale)
    or DVE; PE transposes (bf16, 4-byte-aligned psum cols) rebuild
    P-layout; GRU pointwise runs on [128,5] P-layout tiles.
  - FC2a is split into an hSig-half and an hS-half: the hSig-half MMs
    run DURING the GRU_S phase (its 12 partial rows park in SBUF), so
    the final FC2 phase only runs the hS-half + add + FC2b.  Both
    halves' weights are SBUF-resident; FC2b streams through a ring on
    the scalar DMA queue, interleaved with the hS-pass.
"""

import sys

sys.path.insert(0, "/opt/trn_rl_repo")

import numpy as np
import ml_dtypes

BF16 = ml_dtypes.bfloat16
FP8 = ml_dtypes.float8_e4m3fn
F32 = np.float32
W8SCALE = 32.0

NCORES = 8
H = 576
D2_HID, D2_IN, D2_OUT = 46080, 1152, 576
MSH = D2_HID // NCORES
NM2 = MSH // 128
FCB_GRP = 3

# matvec specs: name -> (seg names, Kp, Mp, act)
MV = {
    "fc5":  (["x5"], 128, 512, "relu"),
    "q_rz": (["out5", "h_q"], 1152, 1280, "sigmoid"),
    "q_in": (["out5"], 512, 640, None),
    "q_hn": (["h_q"], 640, 640, None),
    "fc6":  (["x6"], 128, 512, "relu"),
    "sig_rz": (["hQ", "out6", "h_sig"], 1792, 1280, "sigmoid"),
    "sig_in": (["hQ", "out6"], 1152, 640, None),
    "sig_hn": (["h_sig"], 640, 640, None),
    "fc1":  (["hSig"], 640, 640, "relu"),
    "fc7":  (["obs"], 128, 1024, "relu"),
    "s_rz": (["out1", "out7", "h_s"], 2304, 1280, "sigmoid"),
    "s_in": (["out1", "out7"], 1664, 640, None),
    "s_hn": (["h_s"], 640, 640, None),
}
SEG_COLS = {
    "x5": 1, "x6": 1, "obs": 1,
    "out5": 4, "out6": 4, "out7": 8, "out1": 5,
    "h_q": 5, "h_sig": 5, "h_s": 5,
    "hQ": 5, "hSig": 5, "hS": 5,
}
# columns of the merged const tensor [128, 18]
CONST_COLS = {"x5": (0, 1), "x6": (1, 1), "obs": (2, 1),
              "h_q": (3, 5), "h_sig": (8, 5), "h_s": (13, 5)}

_CACHE = {}


def _stripes(mp):
    return [(n0, min(512, mp - n0)) for n0 in range(0, mp, 512)]


def _build_program():
    import concourse.bass as bass  # noqa: F401
    from concourse import bacc, mybir
    import concourse.tile as tile

    f32 = mybir.dt.float32
    bf16 = mybir.dt.bfloat16
    fp8 = mybir.dt.float8e4
    AF = mybir.ActivationFunctionType

    nc = bacc.Bacc(
        "TRN2", target_bir_lowering=False, debug=False, num_devices=NCORES
    )

    def din(name, shape, dt):
        return nc.dram_tensor(name, list(shape), dt, kind="ExternalInput")

    d_const = din("consts", (128, 18), bf16)
    d_w = {}
    for name, (seg_names, kp, mp, act) in MV.items():
        nk = kp // 128
        for si, (n0, nsz) in enumerate(_stripes(mp)):
            d_w[f"{name}_s{si}"] = din(f"{name}_s{si}", (128, nk, nsz), fp8)
    for si, (n0, nsz) in enumerate(_stripes(MSH)):
        d_w[f"fc2a_sig_s{si}"] = din(f"fc2a_sig_s{si}", (128, 5, nsz), bf16)
        d_w[f"fc2a_hs_s{si}"] = din(f"fc2a_hs_s{si}", (128, 5, nsz), bf16)
    for g in range(NM2 // FCB_GRP):
        d_w[f"fc2b_g{g}"] = din(f"fc2b_g{g}", (128, FCB_GRP, D2_OUT), bf16)
    d_y = nc.dram_tensor("y", [1, D2_OUT], f32, kind="ExternalOutput")

    with tile.TileContext(nc) as tc:
        with (
            tc.tile_pool(name="const", bufs=1) as constp,
            tc.tile_pool(name="vecs", bufs=1) as vecp,
            tc.tile_pool(name="rows", bufs=1) as rowp,
            tc.tile_pool(name="gw", bufs=3) as gwp,
            tc.tile_pool(name="fc2a", bufs=1) as fc2ap,
            tc.tile_pool(name="w2bp", bufs=3) as w2bp,
            tc.tile_pool(name="ps", bufs=1, space="PSUM") as psp,
        ):
            ct = constp.tile([128, 18], bf16, name="t_consts", tag="t_consts")
            nc.sync.dma_start(out=ct, in_=d_const[:])
            # segs: name -> (tile, base col)
            segs = {k: (ct, c0) for k, (c0, _) in CONST_COLS.items()}
            ident = constp.tile([1, 1], bf16, name="ident", tag="ident")
            nc.vector.memset(ident, 1.0)
            # preload ACT LUTs (sigmoid/tanh) while the first weights DMA
            warm = constp.tile([1, 1], f32, name="warm", tag="warm")
            nc.scalar.activation(warm, ident, AF.Sigmoid)
            nc.scalar.activation(warm, ident, AF.Tanh)

            def seg_cols(names):
                cols = []
                for s in names:
                    t, base = segs[s]
                    for j in range(SEG_COLS[s]):
                        cols.append((t, base + j))
                return cols

            def emit_matvec(name):
                seg_names, kp, mp, act = MV[name]
                nk = kp // 128
                cols = seg_cols(seg_names)
                assert len(cols) == nk, (name, len(cols), nk)
                row = rowp.tile([1, mp], bf16, name=f"row_{name}",
                                tag=f"row_{name}")
                for si, (n0, nsz) in enumerate(_stripes(mp)):
                    wt = gwp.tile([128, nk, nsz], fp8, tag="gw",
                                  name=f"w_{name}_{si}", bufs=3)
                    nc.sync.dma_start(out=wt, in_=d_w[f"{name}_s{si}"][:])
                    ps = psp.tile([1, 512], f32, tag="mv", bufs=3,
                                  name=f"ps_{name}_{si}")
                    for c, (st, j) in enumerate(cols):
                        nc.tensor.matmul(
                            ps[0:1, 0:nsz], st[:, j : j + 1],
                            wt[:, c, 0:nsz],
                            start=(c == 0), stop=(c == nk - 1),
                            skip_group_check=True,
                        )
                    dst = row[0:1, n0 : n0 + nsz]
                    if act == "sigmoid":
                        nc.scalar.activation(dst, ps[0:1, 0:nsz], AF.Sigmoid,
                                             scale=1.0 / W8SCALE)
                    elif act == "relu":
                        nc.scalar.activation(dst, ps[0:1, 0:nsz], AF.Relu,
                                             scale=1.0 / W8SCALE)
                    else:
                        nc.vector.tensor_scalar_mul(dst, ps[0:1, 0:nsz],
                                                    1.0 / W8SCALE)
                return row

            def emit_transposes(tp, row, ncols, col0, n_done, n_total):
                for c in range(ncols):
                    nc.tensor.matmul(
                        tp[:, col0 + c, 0:1],
                        row[0:1, c * 128 : (c + 1) * 128], ident,
                        is_transpose=True,
                        start=(n_done + c == 0),
                        stop=(n_done + c == n_total - 1),
                        skip_group_check=True,
                    )
                return n_done + ncols

            def do_fc(name, out_name):
                row = emit_matvec(name)
                nc_ = MV[name][2] // 128
                tp = psp.tile([128, 20, 2], bf16, tag="tp", bufs=2,
                              name=f"tp_{name}")
                emit_transposes(tp, row, nc_, 0, 0, nc_)
                out = vecp.tile([128, nc_], bf16, name=out_name,
                                tag=out_name)
                nc.vector.tensor_copy(out, tp[:, 0:nc_, 0])
                segs[out_name] = (out, 0)

            def do_gru(g, h_name, out_name):
                row_rz = emit_matvec(f"{g}_rz")
                row_in = emit_matvec(f"{g}_in")
                row_hn = emit_matvec(f"{g}_hn")
                tp = psp.tile([128, 20, 2], bf16, tag="tp", bufs=2,
                              name=f"tp_{g}")
                n = emit_transposes(tp, row_rz, 10, 0, 0, 20)
                n = emit_transposes(tp, row_in, 5, 10, n, 20)
                emit_transposes(tp, row_hn, 5, 15, n, 20)
                rzc = vecp.tile([128, 10], bf16, name=f"rzc_{g}", tag="rzc",
                                bufs=2)
                gh = vecp.tile([128, 10], bf16, name=f"gh_{g}", tag="gh",
                               bufs=2)
                nc.vector.tensor_copy(rzc, tp[:, 0:10, 0])
                nc.vector.tensor_copy(gh, tp[:, 10:20, 0])
                t1 = vecp.tile([128, 5], f32, name=f"t1_{g}", tag="t1",
                               bufs=2)
                nt = vecp.tile([128, 5], f32, name=f"nt_{g}", tag="nt",
                               bufs=2)
                t3 = vecp.tile([128, 5], f32, name=f"t3_{g}", tag="t3",
                               bufs=2)
                h_tile, hb = segs[h_name]
                nc.vector.tensor_mul(t1, rzc[:, 0:5], gh[:, 5:10])
                nc.vector.tensor_add(t1, gh[:, 0:5], t1)
                nc.scalar.activation(nt, t1, AF.Tanh)
                nc.vector.tensor_sub(t3, h_tile[:, hb : hb + 5], nt)
                nc.vector.tensor_mul(t3, rzc[:, 5:10], t3)
                out = vecp.tile([128, 5], bf16, name=out_name, tag=out_name)
                nc.vector.tensor_add(out, nt, t3)
                segs[out_name] = (out, 0)

            # ---- the chain ----
            do_fc("fc5", "out5")
            do_gru("q", "h_q", "hQ")
            do_fc("fc6", "out6")
            do_gru("sig", "h_sig", "hSig")
            do_fc("fc1", "out1")
            do_fc("fc7", "out7")

            # ---- FC2a pass 1: hSig half, overlapped with GRU_S ----
            sig_cols = seg_cols(["hSig"])
            partials = []
            for si, (n0, nsz) in enumerate(_stripes(MSH)):
                wt = fc2ap.tile([128, 5, nsz], bf16, name=f"w_fc2as_{si}",
                                tag=f"fc2as_{si}")
                nc.sync.dma_start(out=wt, in_=d_w[f"fc2a_sig_s{si}"][:])
                ps = psp.tile([1, 512], f32, tag="mv", bufs=3,
                              name=f"ps_f1_{si}")
                for c, (st, j) in enumerate(sig_cols):
                    nc.tensor.matmul(
                        ps[0:1, 0:nsz], st[:, j : j + 1], wt[:, c, 0:nsz],
                        start=(c == 0), stop=(c == 4),
                        skip_group_check=True,
                    )
                part = rowp.tile([1, 512], f32, name=f"part_{si}",
                                 tag=f"part_{si}")
                nc.vector.tensor_copy(part[0:1, 0:nsz], ps[0:1, 0:nsz])
                partials.append(part)

            do_gru("s", "h_s", "hS")

            # ---- FC2a pass 2 (hS half) + FC2b, pipelined ----
            hs_cols = seg_cols(["hS"])
            ps_hfc = psp.tile([128, NM2, 2], bf16, tag="hfc", bufs=1,
                              name="ps_hfc")
            h_fc = vecp.tile([128, NM2], bf16, name="h_fc", tag="h_fc")
            ps_y = psp.tile([1, D2_OUT], f32, tag="y", bufs=1, name="ps_y")
            ntp = 0
            fcb_done = 0
            for si, (n0, nsz) in enumerate(_stripes(MSH)):
                wt = fc2ap.tile([128, 5, nsz], bf16, name=f"w_fc2ah_{si}",
                                tag=f"fc2ah_{si}")
                nc.sync.dma_start(out=wt, in_=d_w[f"fc2a_hs_s{si}"][:])
                ps = psp.tile([1, 512], f32, tag="mv", bufs=3,
                              name=f"ps_f2_{si}")
                for c, (st, j) in enumerate(hs_cols):
                    nc.tensor.matmul(
                        ps[0:1, 0:nsz], st[:, j : j + 1], wt[:, c, 0:nsz],
                        start=(c == 0), stop=(c == 4),
                        skip_group_check=True,
                    )
                tmp = rowp.tile([1, 512], f32, name=f"tmp_{si}", tag="tmp",
                                bufs=2)
                nc.vector.tensor_add(tmp[0:1, 0:nsz], ps[0:1, 0:nsz],
                                     partials[si][0:1, 0:nsz])
                hstr = rowp.tile([1, 512], bf16, name=f"hstr_{si}",
                                 tag="hstr", bufs=2)
                nc.scalar.activation(hstr[0:1, 0:nsz], tmp[0:1, 0:nsz],
                                     AF.Relu)
                nb = nsz // 128
                for c in range(nb):
                    nc.tensor.matmul(
                        ps_hfc[:, 4 * si + c, 0:1],
                        hstr[0:1, c * 128 : (c + 1) * 128], ident,
                        is_transpose=True,
                        start=(ntp == 0), stop=(ntp == NM2 - 1),
                        skip_group_check=True,
                    )
                    ntp += 1
                nc.vector.tensor_copy(
                    h_fc[:, 4 * si : 4 * si + nb],
                    ps_hfc[:, 4 * si : 4 * si + nb, 0],
                )
                navail = 4 * si + nb
                while (fcb_done + 1) * FCB_GRP <= navail:
                    gidx = fcb_done
                    wtb = w2bp.tile([128, FCB_GRP, D2_OUT], bf16, tag="w2b",
                                    name=f"w2b_{gidx}", bufs=3)
                    nc.scalar.dma_start(out=wtb, in_=d_w[f"fc2b_g{gidx}"][:])
                    for j in range(FCB_GRP):
                        kb = gidx * FCB_GRP + j
                        lhs = h_fc[:, kb : kb + 1]
                        nc.tensor.matmul(
                            ps_y[0:1, 0:512], lhs, wtb[:, j, 0:512],
                            start=(kb == 0), stop=(kb == NM2 - 1),
                            skip_group_check=True,
                        )
                        nc.tensor.matmul(
                            ps_y[0:1, 512:576], lhs, wtb[:, j, 512:576],
                            start=(kb == 0), stop=(kb == NM2 - 1),
                            skip_group_check=True,
                        )
                    fcb_done += 1
            assert fcb_done == NM2 // FCB_GRP

            y_sb = constp.tile([1, D2_OUT], f32, name="y_sb", tag="y_sb")
            nc.vector.tensor_copy(y_sb, ps_y[0:1, 0:D2_OUT])
            nc.sync.dma_start(out=d_y[:], in_=y_sb)

    nc.compile()
    return nc


def _get_program():
    if "nc" not in _CACHE:
        _CACHE["nc"] = _build_program()
    return _CACHE["nc"]


# ----------------------------------------------------------------------------
# host-side data prep
# ----------------------------------------------------------------------------


def _play_cols(v, ncols, one_slot=None):
    v = np.asarray(v, F32).ravel()
    buf = np.zeros((ncols, 128), F32)
    buf.reshape(-1)[: v.size] = v
    if one_slot is not None:
        buf.reshape(-1)[one_slot] = 1.0
    return buf.T  # [128, ncols] fp32


def _rz_cols(w):
    out = np.zeros((w.shape[0], 1280), F32)
    out[:, 0:576] = w[:, 0:576]
    out[:, 640:1216] = w[:, 576:1152]
    return out


def _swizzle(m, wp, kp, mp, dt=FP8, scale=W8SCALE):
    nk = kp // 128
    w3 = (wp * scale).reshape(nk, 128, mp).transpose(1, 0, 2)
    out = {}
    for si, (n0, nsz) in enumerate(_stripes(mp)):
        out[f"{m}_s{si}"] = np.ascontiguousarray(
            w3[:, :, n0 : n0 + nsz].astype(dt)
        )
    return out


def _gru_host(tag, Wih, Whh, bih, bhh, xsegs):
    out = {}
    kp_rz = sum(p for _, p in xsegs) + 640
    wp = np.zeros((kp_rz, 1280), F32)
    r0 = p0 = 0
    for rows, pad in xsegs:
        wp[p0 : p0 + rows] = _rz_cols(Wih[0:1152, r0 : r0 + rows].T)
        r0 += rows
        p0 += pad
    wp[p0 : p0 + 576] = _rz_cols(Whh[0:1152].T)
    brow = _rz_cols((bih[0:1152] + bhh[0:1152])[None, :])
    brow[0, 1216] = 7.5      # sigmoid(7.5) -> 1.0 in bf16; 7.5*32=240
    # stays below e4m3 exp-15 encodings (>=256), which some decoders
    # treat as inf/nan
    wp[p0 + 576] = brow
    out.update(_swizzle(f"{tag}_rz", wp, kp_rz, 1280))

    kp_in = sum(p for _, p in xsegs)
    wp = np.zeros((kp_in, 640), F32)
    r0 = p0 = 0
    for rows, pad in xsegs:
        wp[p0 : p0 + rows, 0:576] = Wih[1152:1728, r0 : r0 + rows].T
        r0 += rows
        p0 += pad
    wp[xsegs[0][0], 0:576] = bih[1152:1728]
    out.update(_swizzle(f"{tag}_in", wp, kp_in, 640))

    wp = np.zeros((640, 640), F32)
    wp[0:576, 0:576] = Whh[1152:1728].T
    wp[576, 0:576] = bhh[1152:1728]
    out.update(_swizzle(f"{tag}_hn", wp, 640, 640))
    return out


def _fc_host(tag, W, b, kreal, kp, mp, one_col=None):
    wp = np.zeros((kp, mp), F32)
    m = W.shape[0]
    wp[0:kreal, 0:m] = W.T
    wp[kreal, 0:m] = b
    if one_col is not None:
        wp[kreal, one_col] = 1.0
    return _swizzle(tag, wp, kp, mp)


def _prep_inputs(inputs):
    g = {k: np.asarray(v, F32) for k, v in inputs.items()}

    consts = np.zeros((128, 18), F32)
    consts[:, 0:1] = _play_cols(g["fw_evol_diff"], 1, one_slot=24)
    consts[:, 1:2] = _play_cols(g["fw_update_diff"], 1, one_slot=24)
    consts[:, 2:3] = _play_cols(
        np.concatenate([g["obs_diff"], g["obs_innov_diff"]]), 1, one_slot=48
    )
    consts[:, 3:8] = _play_cols(g["h_Q"], 5, one_slot=576)
    consts[:, 8:13] = _play_cols(g["h_Sigma"], 5, one_slot=576)
    consts[:, 13:18] = _play_cols(g["h_S"], 5, one_slot=576)
    common = {"consts": np.ascontiguousarray(consts).astype(BF16)}

    common.update(_fc_host("fc5", g["W5"], g["b5"], 24, 128, 512,
                           one_col=480))
    common.update(_fc_host("fc6", g["W6"], g["b6"], 24, 128, 512,
                           one_col=480))
    common.update(_fc_host("fc7", g["W7"], g["b7"], 48, 128, 1024,
                           one_col=960))
    common.update(_fc_host("fc1", g["W1"], g["b1"], 576, 640, 640,
                           one_col=576))
    common.update(_gru_host("q", g["Wih_Q"], g["Whh_Q"], g["bih_Q"],
                            g["bhh_Q"], [(480, 512)]))
    common.update(_gru_host("sig", g["Wih_Sig"], g["Whh_Sig"],
                            g["bih_Sig"], g["bhh_Sig"],
                            [(576, 640), (480, 512)]))
    common.update(_gru_host("s", g["Wih_S"], g["Whh_S"], g["bih_S"],
                            g["bhh_S"], [(576, 640), (960, 1024)]))

    w2aT = g["W2a"].T
    w2bT = g["W2b"].T
    in_maps = []
    for k in range(NCORES):
        m = dict(common)
        sl = slice(k * MSH, (k + 1) * MSH)
        wp = np.zeros((640, MSH), F32)
        wp[0:576] = w2aT[0:576, sl]
        wp[576] = g["b2a"][sl]
        m.update(_swizzle("fc2a_sig", wp, 640, MSH, dt=BF16, scale=1.0))
        wp = np.zeros((640, MSH), F32)
        wp[0:576] = w2aT[576:1152, sl]
        m.update(_swizzle("fc2a_hs", wp, 640, MSH, dt=BF16, scale=1.0))
        wb3 = w2bT[sl].reshape(NM2, 128, D2_OUT).transpose(1, 0, 2)
        for gi in range(NM2 // FCB_GRP):
            m[f"fc2b_g{gi}"] = np.ascontiguousarray(
                wb3[:, gi * FCB_GRP : (gi + 1) * FCB_GRP, :].astype(BF16)
            )
        in_maps.append(m)
    return in_maps


def run(trace=False, **inputs):
    from concourse.bass_utils import run_bass_kernel_spmd

    nc = _get_program()
    in_maps = _prep_inputs(inputs)
    res = run_bass_kernel_spmd(nc, in_maps, list(range(NCORES)), trace=trace)
    y = np.zeros(D2_OUT, np.float64)
    for r in res.results:
        y += r["y"].reshape(-1).astype(np.float64)
    out = (y.astype(F32) + np.asarray(inputs["b2b"], F32)).reshape(24, 24)
    return out, res


def kernel(**inputs):
    out, _ = run(trace=False, **inputs)
    return out


# revision 16
# speedup vs baseline: 2.4815x; 1.1508x over previous
"""Trainium2 Bass kernel for the KNet-style recurrent chain (batch=1), v3.

Distribution: FC2 tensor-parallel across 8 cores (5760 W2a rows + the
matching W2b columns per core; host sums the 8 partial y's + b2b); the
small GRU/FC chain is replicated on every core.

Memory strategy (the kernel is DMA-bound):
  - GRU/small-FC weights in fp8 e4m3, scaled x32 on host (their ~0.02
    magnitudes would land in e4m3's subnormal range unscaled); the 1/32
    descale is folded into each drain (ACT scale= / DVE scalar-mul).
    Activations stay bf16 (mixed bf16xfp8 matmul).  FC2 stays bf16
    (fp8 FC2 fails the 2e-2 accuracy gate).
  - All weights PRE-SWIZZLED on host into the exact SBUF tile layout
    [128, nk, nsz] so every weight DMA is one contiguous flat copy.
  - All biases are FOLDED INTO THE WEIGHTS via designated always-1.0
    pad slots (slot = element M of each 128-padded vector).  Inputs get
    the 1.0 from the host; FC outputs via relu(1*1); GRU outputs via a
    sigmoid(14)=1 entry in the z-gate pad column (h'[576] = z*h = 1).
  - Matvec psums [1,<=512] drain on ACT (fused sigmoid/relu + descale)
    or DVE; PE transposes (bf16, 4-byte-aligned psum cols) rebuild
    P-layout; GRU pointwise runs on [128,5] P-layout tiles.
  - FC2a is split into an hSig-half and an hS-half: the hSig-half MMs
    run DURING the GRU_S phase (its 12 partial rows park in SBUF), so
    the final FC2 phase only runs the hS-half + add + FC2b.  Both
    halves' weights are SBUF-resident; FC2b streams through a ring on
    the scalar DMA queue, interleaved with the hS-pass.
"""

import sys

sys.path.insert(0, "/opt/trn_rl_repo")

import numpy as np
import ml_dtypes

BF16 = ml_dtypes.bfloat16
FP8 = ml_dtypes.float8_e4m3fn
F32 = np.float32
W8SCALE = 32.0

NCORES = 8
H = 576
D2_HID, D2_IN, D2_OUT = 46080, 1152, 576
MSH = D2_HID // NCORES
NM2 = MSH // 128
FCB_GRP = 3

# matvec specs: name -> (seg names, Kp, Mp, act)
MV = {
    "fc5":  (["x5"], 128, 512, "relu"),
    "q_rz": (["out5", "h_q"], 1152, 1280, "sigmoid"),
    "q_in": (["out5"], 512, 640, None),
    "q_hn": (["h_q"], 640, 640, None),
    "fc6":  (["x6"], 128, 512, "relu"),
    "sig_rz": (["hQ", "out6", "h_sig"], 1792, 1280, "sigmoid"),
    "sig_in": (["hQ", "out6"], 1152, 640, None),
    "sig_hn": (["h_sig"], 640, 640, None),
    "fc1":  (["hSig"], 640, 640, "relu"),
    "fc7":  (["obs"], 128, 1024, "relu"),
    "s_rz": (["out1", "out7", "h_s"], 2304, 1280, "sigmoid"),
    "s_in": (["out1", "out7"], 1664, 640, None),
    "s_hn": (["h_s"], 640, 640, None),
}
SEG_COLS = {
    "x5": 1, "x6": 1, "obs": 1,
    "out5": 4, "out6": 4, "out7": 8, "out1": 5,
    "h_q": 5, "h_sig": 5, "h_s": 5,
    "hQ": 5, "hSig": 5, "hS": 5,
}
# columns of the merged const tensor [128, 18]
CONST_COLS = {"x5": (0, 1), "x6": (1, 1), "obs": (2, 1),
              "h_q": (3, 5), "h_sig": (8, 5), "h_s": (13, 5)}

_CACHE = {}


def _stripes(mp):
    return [(n0, min(512, mp - n0)) for n0 in range(0, mp, 512)]


def _build_program():
    import concourse.bass as bass  # noqa: F401
    from concourse import bacc, mybir
    import concourse.tile as tile

    f32 = mybir.dt.float32
    bf16 = mybir.dt.bfloat16
    fp8 = mybir.dt.float8e4
    AF = mybir.ActivationFunctionType

    nc = bacc.Bacc(
        "TRN2", target_bir_lowering=False, debug=False, num_devices=NCORES
    )

    def din(name, shape, dt):
        return nc.dram_tensor(name, list(shape), dt, kind="ExternalInput")

    d_const = din("consts", (128, 18), bf16)
    d_w = {}
    for name, (seg_names, kp, mp, act) in MV.items():
        nk = kp // 128
        for si, (n0, nsz) in enumerate(_stripes(mp)):
            d_w[f"{name}_s{si}"] = din(f"{name}_s{si}", (128, nk, nsz), fp8)
    for si, (n0, nsz) in enumerate(_stripes(MSH)):
        d_w[f"fc2a_sig_s{si}"] = din(f"fc2a_sig_s{si}", (128, 5, nsz), bf16)
        d_w[f"fc2a_hs_s{si}"] = din(f"fc2a_hs_s{si}", (128, 5, nsz), bf16)
    for g in range(NM2 // FCB_GRP):
        d_w[f"fc2b_g{g}"] = din(f"fc2b_g{g}", (128, FCB_GRP, D2_OUT), bf16)
    d_y = nc.dram_tensor("y", [1, D2_OUT], f32, kind="ExternalOutput")

    with tile.TileContext(nc) as tc:
        with (
            tc.tile_pool(name="const", bufs=1) as constp,
            tc.tile_pool(name="vecs", bufs=1) as vecp,
            tc.tile_pool(name="rows", bufs=1) as rowp,
            tc.tile_pool(name="gw", bufs=6) as gwp,
            tc.tile_pool(name="fc2a", bufs=1) as fc2ap,
            tc.tile_pool(name="w2bp", bufs=3) as w2bp,
            tc.tile_pool(name="ps", bufs=1, space="PSUM") as psp,
        ):
            ct = constp.tile([128, 18], bf16, name="t_consts", tag="t_consts")
            nc.sync.dma_start(out=ct, in_=d_const[:])
            # segs: name -> (tile, base col)
            segs = {k: (ct, c0) for k, (c0, _) in CONST_COLS.items()}
            ident = constp.tile([1, 1], bf16, name="ident", tag="ident")
            nc.vector.memset(ident, 1.0)
            # preload ACT LUTs (sigmoid/tanh) while the first weights DMA
            warm = constp.tile([1, 1], f32, name="warm", tag="warm")
            nc.scalar.activation(warm, ident, AF.Sigmoid)
            nc.scalar.activation(warm, ident, AF.Tanh)

            # FC2a resident tiles; triggers go on the SCALAR dma queue,
            # woven through the chain, so a stalled GRU-ring trigger on
            # the sync queue never blocks FC2a prefetch (head-of-line).
            fc2a_sig_tiles = []
            fc2a_hs_tiles = []
            _sig_pending = list(enumerate(_stripes(MSH)))
            _hs_pending = list(enumerate(_stripes(MSH)))

            def drop_sig(n):
                for _ in range(n):
                    if not _sig_pending:
                        return
                    si, (n0, nsz) = _sig_pending.pop(0)
                    t = fc2ap.tile([128, 5, nsz], bf16,
                                   name=f"w_fc2as_{si}", tag=f"fc2as_{si}")
                    nc.scalar.dma_start(out=t, in_=d_w[f"fc2a_sig_s{si}"][:])
                    fc2a_sig_tiles.append(t)

            def drop_hs(n):
                for _ in range(n):
                    if not _hs_pending:
                        return
                    si, (n0, nsz) = _hs_pending.pop(0)
                    t = fc2ap.tile([128, 5, nsz], bf16,
                                   name=f"w_fc2ah_{si}", tag=f"fc2ah_{si}")
                    nc.scalar.dma_start(out=t, in_=d_w[f"fc2a_hs_s{si}"][:])
                    fc2a_hs_tiles.append(t)

            def seg_cols(names):
                cols = []
                for s in names:
                    t, base = segs[s]
                    for j in range(SEG_COLS[s]):
                        cols.append((t, base + j))
                return cols

            def emit_matvec(name):
                seg_names, kp, mp, act = MV[name]
                nk = kp // 128
                cols = seg_cols(seg_names)
                assert len(cols) == nk, (name, len(cols), nk)
                row = rowp.tile([1, mp], bf16, name=f"row_{name}",
                                tag=f"row_{name}")
                for si, (n0, nsz) in enumerate(_stripes(mp)):
                    wt = gwp.tile([128, nk, nsz], fp8, tag="gw",
                                  name=f"w_{name}_{si}", bufs=3)
                    nc.sync.dma_start(out=wt, in_=d_w[f"{name}_s{si}"][:])
                    ps = psp.tile([1, 512], f32, tag="mv", bufs=3,
                                  name=f"ps_{name}_{si}")
                    for c, (st, j) in enumerate(cols):
                        nc.tensor.matmul(
                            ps[0:1, 0:nsz], st[:, j : j + 1],
                            wt[:, c, 0:nsz],
                            start=(c == 0), stop=(c == nk - 1),
                            skip_group_check=True,
                        )
                    dst = row[0:1, n0 : n0 + nsz]
                    if act == "sigmoid":
                        nc.scalar.activation(dst, ps[0:1, 0:nsz], AF.Sigmoid,
                                             scale=1.0 / W8SCALE)
                    elif act == "relu":
                        nc.scalar.activation(dst, ps[0:1, 0:nsz], AF.Relu,
                                             scale=1.0 / W8SCALE)
                    else:
                        nc.vector.tensor_scalar_mul(dst, ps[0:1, 0:nsz],
                                                    1.0 / W8SCALE)
                return row

            def emit_transposes(tp, row, ncols, col0, n_done, n_total):
                for c in range(ncols):
                    nc.tensor.matmul(
                        tp[:, col0 + c, 0:1],
                        row[0:1, c * 128 : (c + 1) * 128], ident,
                        is_transpose=True,
                        start=(n_done + c == 0),
                        stop=(n_done + c == n_total - 1),
                        skip_group_check=True,
                    )
                return n_done + ncols

            def do_fc(name, out_name):
                row = emit_matvec(name)
                nc_ = MV[name][2] // 128
                tp = psp.tile([128, 20, 2], bf16, tag="tp", bufs=2,
                              name=f"tp_{name}")
                emit_transposes(tp, row, nc_, 0, 0, nc_)
                out = vecp.tile([128, nc_], bf16, name=out_name,
                                tag=out_name)
                nc.vector.tensor_copy(out, tp[:, 0:nc_, 0])
                segs[out_name] = (out, 0)

            def do_gru(g, h_name, out_name):
                row_rz = emit_matvec(f"{g}_rz")
                row_in = emit_matvec(f"{g}_in")
                row_hn = emit_matvec(f"{g}_hn")
                tp = psp.tile([128, 20, 2], bf16, tag="tp", bufs=2,
                              name=f"tp_{g}")
                n = emit_transposes(tp, row_rz, 10, 0, 0, 20)
                n = emit_transposes(tp, row_in, 5, 10, n, 20)
                emit_transposes(tp, row_hn, 5, 15, n, 20)
                rzc = vecp.tile([128, 10], bf16, name=f"rzc_{g}", tag="rzc",
                                bufs=2)
                gh = vecp.tile([128, 10], bf16, name=f"gh_{g}", tag="gh",
                               bufs=2)
                nc.vector.tensor_copy(rzc, tp[:, 0:10, 0])
                nc.vector.tensor_copy(gh, tp[:, 10:20, 0])
                t1 = vecp.tile([128, 5], f32, name=f"t1_{g}", tag="t1",
                               bufs=2)
                nt = vecp.tile([128, 5], f32, name=f"nt_{g}", tag="nt",
                               bufs=2)
                t3 = vecp.tile([128, 5], f32, name=f"t3_{g}", tag="t3",
                               bufs=2)
                h_tile, hb = segs[h_name]
                nc.vector.tensor_mul(t1, rzc[:, 0:5], gh[:, 5:10])
                nc.vector.tensor_add(t1, gh[:, 0:5], t1)
                nc.scalar.activation(nt, t1, AF.Tanh)
                nc.vector.tensor_sub(t3, h_tile[:, hb : hb + 5], nt)
                nc.vector.tensor_mul(t3, rzc[:, 5:10], t3)
                out = vecp.tile([128, 5], bf16, name=out_name, tag=out_name)
                nc.vector.tensor_add(out, nt, t3)
                segs[out_name] = (out, 0)

            # ---- the chain ----
            do_fc("fc5", "out5")
            drop_sig(2)
            do_gru("q", "h_q", "hQ")
            drop_sig(3)
            do_fc("fc6", "out6")
            drop_sig(2)
            do_gru("sig", "h_sig", "hSig")
            drop_sig(3)
            do_fc("fc1", "out1")
            drop_sig(2)
            do_fc("fc7", "out7")
            drop_hs(4)
            assert not _sig_pending

            # ---- FC2a pass 1: hSig half, overlapped with GRU_S ----
            sig_cols = seg_cols(["hSig"])
            partials = []
            for si, (n0, nsz) in enumerate(_stripes(MSH)):
                drop_hs(1)
                wt = fc2a_sig_tiles[si]
                ps = psp.tile([1, 512], f32, tag="mv", bufs=3,
                              name=f"ps_f1_{si}")
                for c, (st, j) in enumerate(sig_cols):
                    nc.tensor.matmul(
                        ps[0:1, 0:nsz], st[:, j : j + 1], wt[:, c, 0:nsz],
                        start=(c == 0), stop=(c == 4),
                        skip_group_check=True,
                    )
                part = rowp.tile([1, 512], f32, name=f"part_{si}",
                                 tag=f"part_{si}")
                nc.vector.tensor_copy(part[0:1, 0:nsz], ps[0:1, 0:nsz])
                partials.append(part)

            do_gru("s", "h_s", "hS")
            assert not _hs_pending

            # ---- FC2a pass 2 (hS half) + FC2b, pipelined ----
            hs_cols = seg_cols(["hS"])
            ps_hfc = psp.tile([128, NM2, 2], bf16, tag="hfc", bufs=1,
                              name="ps_hfc")
            h_fc = vecp.tile([128, NM2], bf16, name="h_fc", tag="h_fc")
            ps_y = psp.tile([1, D2_OUT], f32, tag="y", bufs=1, name="ps_y")
            ntp = 0
            fcb_done = 0
            for si, (n0, nsz) in enumerate(_stripes(MSH)):
                wt = fc2a_hs_tiles[si]
                ps = psp.tile([1, 512], f32, tag="mv", bufs=3,
                              name=f"ps_f2_{si}")
                for c, (st, j) in enumerate(hs_cols):
                    nc.tensor.matmul(
                        ps[0:1, 0:nsz], st[:, j : j + 1], wt[:, c, 0:nsz],
                        start=(c == 0), stop=(c == 4),
                        skip_group_check=True,
                    )
                tmp = rowp.tile([1, 512], f32, name=f"tmp_{si}", tag="tmp",
                                bufs=2)
                nc.vector.tensor_add(tmp[0:1, 0:nsz], ps[0:1, 0:nsz],
                                     partials[si][0:1, 0:nsz])
                hstr = rowp.tile([1, 512], bf16, name=f"hstr_{si}",
                                 tag="hstr", bufs=2)
                nc.scalar.activation(hstr[0:1, 0:nsz], tmp[0:1, 0:nsz],
                                     AF.Relu)
                nb = nsz // 128
                for c in range(nb):
                    nc.tensor.matmul(
                        ps_hfc[:, 4 * si + c, 0:1],
                        hstr[0:1, c * 128 : (c + 1) * 128], ident,
                        is_transpose=True,
                        start=(ntp == 0), stop=(ntp == NM2 - 1),
                        skip_group_check=True,
                    )
                    ntp += 1
                nc.vector.tensor_copy(
                    h_fc[:, 4 * si : 4 * si + nb],
                    ps_hfc[:, 4 * si : 4 * si + nb, 0],
                )
                navail = 4 * si + nb
                while (fcb_done + 1) * FCB_GRP <= navail:
                    gidx = fcb_done
                    wtb = w2bp.tile([128, FCB_GRP, D2_OUT], bf16, tag="w2b",
                                    name=f"w2b_{gidx}", bufs=3)
                    nc.scalar.dma_start(out=wtb, in_=d_w[f"fc2b_g{gidx}"][:])
                    for j in range(FCB_GRP):
                        kb = gidx * FCB_GRP + j
                        lhs = h_fc[:, kb : kb + 1]
                        nc.tensor.matmul(
                            ps_y[0:1, 0:512], lhs, wtb[:, j, 0:512],
                            start=(kb == 0), stop=(kb == NM2 - 1),
                            skip_group_check=True,
                        )
                        nc.tensor.matmul(
                            ps_y[0:1, 512:576], lhs, wtb[:, j, 512:576],
                            start=(kb == 0), stop=(kb == NM2 - 1),
                            skip_group_check=True,
                        )
                    fcb_done += 1
            assert fcb_done == NM2 // FCB_GRP

            y_sb = constp.tile([1, D2_OUT], f32, name="y_sb", tag="y_sb")
            nc.vector.tensor_copy(y_sb, ps_y[0:1, 0:D2_OUT])
            nc.sync.dma_start(out=d_y[:], in_=y_sb)

    nc.compile()
    return nc


def _get_program():
    if "nc" not in _CACHE:
        _CACHE["nc"] = _build_program()
    return _CACHE["nc"]


# ----------------------------------------------------------------------------
# host-side data prep
# ----------------------------------------------------------------------------


def _play_cols(v, ncols, one_slot=None):
    v = np.asarray(v, F32).ravel()
    buf = np.zeros((ncols, 128), F32)
    buf.reshape(-1)[: v.size] = v
    if one_slot is not None:
        buf.reshape(-1)[one_slot] = 1.0
    return buf.T  # [128, ncols] fp32


def _rz_cols(w):
    out = np.zeros((w.shape[0], 1280), F32)
    out[:, 0:576] = w[:, 0:576]
    out[:, 640:1216] = w[:, 576:1152]
    return out


def _swizzle(m, wp, kp, mp, dt=FP8, scale=W8SCALE):
    nk = kp // 128
    w3 = (wp * scale).reshape(nk, 128, mp).transpose(1, 0, 2)
    out = {}
    for si, (n0, nsz) in enumerate(_stripes(mp)):
        out[f"{m}_s{si}"] = np.ascontiguousarray(
            w3[:, :, n0 : n0 + nsz].astype(dt)
        )
    return out


def _gru_host(tag, Wih, Whh, bih, bhh, xsegs):
    out = {}
    kp_rz = sum(p for _, p in xsegs) + 640
    wp = np.zeros((kp_rz, 1280), F32)
    r0 = p0 = 0
    for rows, pad in xsegs:
        wp[p0 : p0 + rows] = _rz_cols(Wih[0:1152, r0 : r0 + rows].T)
        r0 += rows
        p0 += pad
    wp[p0 : p0 + 576] = _rz_cols(Whh[0:1152].T)
    brow = _rz_cols((bih[0:1152] + bhh[0:1152])[None, :])
    brow[0, 1216] = 7.5      # sigmoid(7.5) -> 1.0 in bf16; 7.5*32=240
    # stays below e4m3 exp-15 encodings (>=256), which some decoders
    # treat as inf/nan
    wp[p0 + 576] = brow
    out.update(_swizzle(f"{tag}_rz", wp, kp_rz, 1280))

    kp_in = sum(p for _, p in xsegs)
    wp = np.zeros((kp_in, 640), F32)
    r0 = p0 = 0
    for rows, pad in xsegs:
        wp[p0 : p0 + rows, 0:576] = Wih[1152:1728, r0 : r0 + rows].T
        r0 += rows
        p0 += pad
    wp[xsegs[0][0], 0:576] = bih[1152:1728]
    out.update(_swizzle(f"{tag}_in", wp, kp_in, 640))

    wp = np.zeros((640, 640), F32)
    wp[0:576, 0:576] = Whh[1152:1728].T
    wp[576, 0:576] = bhh[1152:1728]
    out.update(_swizzle(f"{tag}_hn", wp, 640, 640))
    return out


def _fc_host(tag, W, b, kreal, kp, mp, one_col=None):
    wp = np.zeros((kp, mp), F32)
    m = W.shape[0]
    wp[0:kreal, 0:m] = W.T
    wp[kreal, 0:m] = b
    if one_col is not None:
        wp[kreal, one_col] = 1.0
    return _swizzle(tag, wp, kp, mp)


def _prep_inputs(inputs):
    g = {k: np.asarray(v, F32) for k, v in inputs.items()}

    consts = np.zeros((128, 18), F32)
    consts[:, 0:1] = _play_cols(g["fw_evol_diff"], 1, one_slot=24)
    consts[:, 1:2] = _play_cols(g["fw_update_diff"], 1, one_slot=24)
    consts[:, 2:3] = _play_cols(
        np.concatenate([g["obs_diff"], g["obs_innov_diff"]]), 1, one_slot=48
    )
    consts[:, 3:8] = _play_cols(g["h_Q"], 5, one_slot=576)
    consts[:, 8:13] = _play_cols(g["h_Sigma"], 5, one_slot=576)
    consts[:, 13:18] = _play_cols(g["h_S"], 5, one_slot=576)
    common = {"consts": np.ascontiguousarray(consts).astype(BF16)}

    common.update(_fc_host("fc5", g["W5"], g["b5"], 24, 128, 512,
                           one_col=480))
    common.update(_fc_host("fc6", g["W6"], g["b6"], 24, 128, 512,
                           one_col=480))
    common.update(_fc_host("fc7", g["W7"], g["b7"], 48, 128, 1024,
                           one_col=960))
    common.update(_fc_host("fc1", g["W1"], g["b1"], 576, 640, 640,
                           one_col=576))
    common.update(_gru_host("q", g["Wih_Q"], g["Whh_Q"], g["bih_Q"],
                            g["bhh_Q"], [(480, 512)]))
    common.update(_gru_host("sig", g["Wih_Sig"], g["Whh_Sig"],
                            g["bih_Sig"], g["bhh_Sig"],
                            [(576, 640), (480, 512)]))
    common.update(_gru_host("s", g["Wih_S"], g["Whh_S"], g["bih_S"],
                            g["bhh_S"], [(576, 640), (960, 1024)]))

    w2aT = g["W2a"].T
    w2bT = g["W2b"].T
    in_maps = []
    for k in range(NCORES):
        m = dict(common)
        sl = slice(k * MSH, (k + 1) * MSH)
        wp = np.zeros((640, MSH), F32)
        wp[0:576] = w2aT[0:576, sl]
        wp[576] = g["b2a"][sl]
        m.update(_swizzle("fc2a_sig", wp, 640, MSH, dt=BF16, scale=1.0))
        wp = np.zeros((640, MSH), F32)
        wp[0:576] = w2aT[576:1152, sl]
        m.update(_swizzle("fc2a_hs", wp, 640, MSH, dt=BF16, scale=1.0))
        wb3 = w2bT[sl].reshape(NM2, 128, D2_OUT).transpose(1, 0, 2)
        for gi in range(NM2 // FCB_GRP):
            m[f"fc2b_g{gi}"] = np.ascontiguousarray(
                wb3[:, gi * FCB_GRP : (gi + 1) * FCB_GRP, :].astype(BF16)
            )
        in_maps.append(m)
    return in_maps


def run(trace=False, **inputs):
    from concourse.bass_utils import run_bass_kernel_spmd

    nc = _get_program()
    in_maps = _prep_inputs(inputs)
    res = run_bass_kernel_spmd(nc, in_maps, list(range(NCORES)), trace=trace)
    y = np.zeros(D2_OUT, np.float64)
    for r in res.results:
        y += r["y"].reshape(-1).astype(np.float64)
    out = (y.astype(F32) + np.asarray(inputs["b2b"], F32)).reshape(24, 24)
    return out, res


def kernel(**inputs):
    out, _ = run(trace=False, **inputs)
    return out
